# revision 15
# baseline (speedup 1.0000x reference)
"""CAB multi-head attention on 8 Trainium2 NeuronCores.

Sharding: fully data-parallel, core c -> (batch b = c//2, query-half = c%2).
Each core computes 256 query rows against all 512 keys of its batch.
No collectives. Host does transposes/packing; device does all FLOPs.

Per-core layout conventions (features on partitions, tokens on free):
  QT/KT [E, t] bf16; V [s, e] bf16; scoresT/attnT [s, t] (softmax along
  partitions via one-hot-column matmuls, no max subtraction needed);
  CAB pairs i-major: h/h2 [(d, i%2), j]; comp [(iic, i%2+h), j] is
  PE-transposed into biasT [j, (tt, jc, c)] and pre-loaded into the
  scores PSUM via an identity matmul with a strided moving AP.

All matmuls bf16 (fp32 PSUM accumulate). head_temps folded into W3 on
host; b3*temps added via the exp activation bias. Projection weights
host-packed so every DMA is contiguous. Phase-1 projections interleaved
into the CAB tt loop so PE/DVE/Scalar all stay busy while weights
stream in.
"""
import sys

sys.path.insert(0, "/opt/trn_rl_repo")

import numpy as np
import ml_dtypes
from contextlib import ExitStack

import concourse.bacc as bacc
import concourse.tile as tile
from concourse import mybir
from concourse.bass_utils import run_bass_kernel_spmd

F32 = mybir.dt.float32
F32R = mybir.dt.float32r
BF16 = mybir.dt.bfloat16
AF = mybir.ActivationFunctionType
ALU = mybir.AluOpType

B, N, E, H, SD, HID = 4, 512, 1024, 16, 64, 64
D = E // H
NQ = 256            # query rows per core
NCORES = 8
NTT = NQ // 8       # 32 tt groups (4 i-pairs each) in the CAB stage

_BF = ml_dtypes.bfloat16


def _build_program():
    nc = bacc.Bacc("TRN2", target_bir_lowering=False, debug=False,
                   num_devices=NCORES)

    def din(name, shape, dt):
        return nc.dram_tensor(name, list(shape), dt, kind="ExternalInput").ap()

    d = {}
    # packed consts
    d["f32c"] = din("f32c", (128, 34), F32)      # bq(8) bk(8) b1(1) b2(1) b3t(16)
    d["bf16c"] = din("bf16c", (128, 672), BF16)  # id128 hsel w2bd w3bd idtr
    d["w1ab"] = din("w1ab", (SD, 256), BF16)     # w1a | w1b
    d["seT"] = din("seT", (SD, N), BF16)
    d["seQ"] = din("seQ", (SD, NQ), BF16)
    d["sel8"] = din("sel8", (16, 1024), F32R)    # recip row-broadcast selectors
    d["onespk"] = din("onespk", (1, 128 + E), BF16)  # ones(128) | bv
    # per-core activations
    d["qT"] = din("qT", (E, NQ), BF16)
    d["kT"] = din("kT", (E, N), BF16)
    d["vtp"] = din("vtp", (N, E), BF16)          # packed V-proj stationary
    # weights (packed/bf16)
    d["wqp"] = din("wqp", (E, E), BF16)
    d["wkp"] = din("wkp", (E, E), BF16)
    d["wv"] = din("wv", (E, E), BF16)
    d["wo"] = din("wo", (E, E), BF16)
    out_d = nc.dram_tensor("out", [NQ, E], F32, kind="ExternalOutput").ap()

    with tile.TileContext(nc) as tc, ExitStack() as ctx:
        # ---------------- persistent SBUF ----------------
        cst = ctx.enter_context(tc.tile_pool(name="cst", bufs=1))
        big = ctx.enter_context(tc.tile_pool(name="big", bufs=1))

        def cload(name, shape, dt, src=None):
            t = cst.tile(list(shape), dt, tag=name, name=name)
            nc.sync.dma_start(t[:], (src if src is not None else d[name][:]))
            return t

        # small consts first (CAB needs them immediately)
        f32c = cload("f32c", (128, 34), F32)
        bq128 = f32c[:, 0:8]
        bk128 = f32c[:, 8:16]
        b1d = f32c[:, 16:17]
        b2d = f32c[:, 17:18]
        b3t128 = f32c[:, 18:34]
        bf16c = cload("bf16c", (128, 672), BF16)
        id128 = bf16c[:, 0:128]
        hsel = bf16c[:, 128:384]
        w2bd = bf16c[:, 384:512]
        w3bd = bf16c[:, 512:544]
        idtr = bf16c[:, 544:672]
        w1ab = cload("w1ab", (SD, 256), BF16)
        seQ = cload("seQ", (SD, NQ), BF16)
        seT = cload("seT", (SD, N), BF16)
        sel8 = cload("sel8", (16, 1024), F32R)
        onespk = cload("onespk", (1, 128 + E), BF16)
        ones1 = onespk[:, 0:128]
        bv2d = onespk[:, 128:128 + E]

        # big per-core inputs / weights, one contiguous DMA each, in
        # priority order (CAB first, out-proj weights last).
        def kview(name, t, chunk_elems):
            tt = big.tile([128, t], BF16, tag=name, name=name)
            nc.sync.dma_start(
                tt[:], d[name][:].rearrange("(k p) t -> p k t", p=128))
            return tt

        kTt = kview("kT", 8 * N, N)            # [128, (k s)]
        vtp = kview("vtp", 4 * E, E)           # [128, (st kc)]
        qTt = kview("qT", 8 * NQ, NQ)          # [128, (k t)]
        wv_r = kview("wv", 8 * E, E)
        wkp = kview("wkp", 8 * E, E)
        wqp = kview("wqp", 8 * E, E)
        wo_r = kview("wo", 8 * E, E)

        def kc(tbl, k, w):
            return tbl[:, k * w:(k + 1) * w]

        # persistent intermediates
        QT = big.tile([128, 8 * NQ], BF16, tag="QT")
        KT = big.tile([128, 8 * N], BF16, tag="KT")
        Vsb = big.tile([128, 8 * 512], BF16, tag="Vsb")   # (st et) chunks
        hjT = big.tile([128, N], BF16, tag="hjT")
        hiT = big.tile([128, 128], F32, tag="hiT")
        biasT = big.tile([128, NTT * 512], BF16, tag="biasT")
        avU = big.tile([128, 8 * NQ], F32, tag="avU")
        avN = big.tile([128, 8 * NQ], BF16, tag="avN")
        sums_h = big.tile([16, NQ], F32, tag="sums_h")
        sums_sb = big.tile([16, NQ], F32, tag="sums_sb")
        recip_sb = big.tile([16, NQ], F32R, tag="recip_sb")

        # ---------------- phase A: CAB MLP with projections woven in ----
        with tc.tile_pool(name="hpool", bufs=6) as hpool, \
             tc.tile_pool(name="h2sb", bufs=3) as h2sbp, \
             tc.tile_pool(name="csb", bufs=3) as csbp, \
             tc.tile_pool(name="w2ps", bufs=2, space="PSUM") as w2ps, \
             tc.tile_pool(name="cps", bufs=2, space="PSUM") as cps, \
             tc.tile_pool(name="trps", bufs=1, space="PSUM") as trps, \
             tc.tile_pool(name="p1ps", bufs=1, space="PSUM") as p1ps:

            # W1: hjT (dup'd, +b1, bf16) and hiT (packed by i-parity, f32)
            hj_ps = p1ps.tile([128, N], F32, tag="p1")
            nc.tensor.matmul(hj_ps[:], w1ab[:, 128:256], seT[:],
                             start=True, stop=True)
            nc.vector.tensor_scalar(hjT[:], hj_ps[:], b1d[:, 0:1], None,
                                    ALU.add)
            hi_ps = p1ps.tile([128, N], F32, tag="p1", name="hi_ps")[:, 0:NQ]
            nc.tensor.matmul(hi_ps[:], w1ab[:, 0:128], seQ[:],
                             start=True, stop=True)
            hi_v = hi_ps[:].rearrange("p (i two) -> p i two", two=2)
            nc.vector.tensor_copy(hiT[0:64, :], hi_v[0:64, :, 0])
            nc.vector.tensor_copy(hiT[64:128, :], hi_v[64:128, :, 1])

            # phase-1 chunk emitters (interleaved into the tt loop)
            def v_chunk(i):
                st, et = i // 2, i % 2
                ps = p1ps.tile([128, N], F32, tag="p1")
                for k in range(8):
                    nc.tensor.matmul(
                        ps[:], kc(vtp, st, E)[:, k * 128:(k + 1) * 128],
                        kc(wv_r, k, E)[:, et * 512:(et + 1) * 512],
                        start=(k == 0), stop=False)
                nc.tensor.matmul(ps[:], ones1[0:1, 0:128],
                                 bv2d[0:1, et * 512:(et + 1) * 512],
                                 start=False, stop=True)
                nc.scalar.copy(Vsb[:, (st * 2 + et) * 512:
                                   (st * 2 + et + 1) * 512], ps[:])

            def k_chunk(ec):
                ps = p1ps.tile([128, N], F32, tag="p1")
                for k in range(8):
                    nc.tensor.matmul(
                        ps[:], kc(wkp, ec, E)[:, k * 128:(k + 1) * 128],
                        kc(kTt, k, N), start=(k == 0), stop=(k == 7))
                nc.vector.tensor_scalar(kc(KT, ec, N), ps[:],
                                        bk128[:, ec:ec + 1], None, ALU.add)

            def q_chunk(ec):
                ps = p1ps.tile([128, N], F32, tag="p1", name="qps")[:, 0:NQ]
                for k in range(8):
                    nc.tensor.matmul(
                        ps[:], kc(wqp, ec, E)[:, k * 128:(k + 1) * 128],
                        kc(qTt, k, NQ), start=(k == 0), stop=(k == 7))
                nc.vector.tensor_scalar(kc(QT, ec, NQ), ps[:],
                                        bq128[:, ec:ec + 1], None, ALU.add)

            sched = {}
            for n, i in enumerate(range(8)):          # V at tt 2..9
                sched.setdefault(2 + n, []).append(("v", i))
            for n, ec in enumerate(range(8)):         # K at tt 11,13..25
                sched.setdefault(11 + 2 * n, []).append(("k", ec))
            for n, ec in enumerate(range(8)):         # Q at tt 24..31
                sched.setdefault(24 + n, []).append(("q", ec))

            for tt in range(NTT):
                h_tiles = []
                for iic in range(4):
                    ii = tt * 4 + iic
                    h_t = hpool.tile([128, N], BF16, tag="h")
                    nc.vector.tensor_scalar(h_t[:], hjT[:],
                                            hiT[:, ii:ii + 1], 0.0,
                                            ALU.add, ALU.max)
                    h_tiles.append(h_t)
                h2_tiles = []
                for half in range(2):
                    ps2 = w2ps.tile([128, 2 * N], F32, tag="h2")
                    for j in range(2):
                        nc.tensor.matmul(ps2[:, j * N:(j + 1) * N], w2bd[:],
                                         h_tiles[half * 2 + j][:],
                                         start=True, stop=True)
                    h2_t = h2sbp.tile([128, 2 * N], BF16, tag="h2sb")
                    nc.scalar.activation(h2_t[:], ps2[:], AF.Relu,
                                         bias=b2d[:, 0:1])
                    h2_tiles.append(h2_t)

                c_ps = cps.tile([128, N], F32, tag="comp")
                for iic in range(4):
                    nc.tensor.matmul(c_ps[32 * iic:32 * iic + 32, :],
                                     w3bd[:],
                                     h2_tiles[iic // 2][:, (iic % 2) * N:
                                                        (iic % 2 + 1) * N],
                                     start=True, stop=True,
                                     tile_position=(0, 32 * iic))
                c_sb = csbp.tile([128, N], BF16, tag="csb")
                if tt % 2 == 0:
                    nc.vector.tensor_copy(c_sb[:], c_ps[:])
                else:
                    nc.scalar.copy(c_sb[:], c_ps[:])
                tr_ps = trps.tile([128, 512], BF16, tag="tr")
                for jc in range(4):
                    nc.tensor.transpose(tr_ps[:, jc * 128:(jc + 1) * 128],
                                        c_sb[:, jc * 128:(jc + 1) * 128],
                                        idtr[:])
                # tr_ps cols are (jc, h, iic, ipar) via the permuted
                # identity; copy into h-major biasT (col = h*1024 +
                # jc*256 + t) so phase-B preloads are contiguous.
                bdst = biasT[:].rearrange(
                    "p (x j to i m) -> p j x i m to",
                    x=16, j=4, to=NTT, i=4, m=2)[:, :, :, :, :, tt]
                bsrc = tr_ps[:].rearrange(
                    "p (j x i m) -> p j x i m", j=4, x=16, i=4, m=2)
                nc.vector.tensor_copy(bdst, bsrc)

                for kind, i in sched.get(tt, []):
                    (v_chunk if kind == "v" else
                     k_chunk if kind == "k" else q_chunk)(i)

        # ---------------- phase B: scores + softmax + AV ----------------
        with tc.tile_pool(name="attnT", bufs=8) as attp, \
             tc.tile_pool(name="scps", bufs=5, space="PSUM") as scps, \
             tc.tile_pool(name="smps", bufs=1, space="PSUM") as smps, \
             tc.tile_pool(name="avps", bufs=2, space="PSUM") as avps:

            sums_ps = smps.tile([16, 2 * NQ], F32, tag="sums")
            av_tiles = [None] * 8
            for h in range(16):
                hp, hw = h // 2, (h % 2) * 64
                if h % 2 == 0:
                    av_ps = avps.tile([128, NQ], F32, tag="av")
                    av_tiles[hp] = av_ps
                at_h = []
                for jcp in range(2):           # two j-chunks per tile
                    sc_ps = scps.tile([128, 2 * NQ], F32, tag="sc")
                    for jh in range(2):
                        jc = jcp * 2 + jh
                        nc.tensor.matmul(
                            sc_ps[:, jh * NQ:(jh + 1) * NQ],
                            kc(KT, hp, N)[hw:hw + 64,
                                          jc * 128:(jc + 1) * 128],
                            kc(QT, hp, NQ)[hw:hw + 64, :],
                            start=True, stop=True, skip_group_check=True)
                    # bias add on DVE (PE stays matmul-only); exp reads f32
                    nc.vector.tensor_tensor(
                        sc_ps[:], sc_ps[:],
                        biasT[:, h * 1024 + jcp * 512:
                              h * 1024 + (jcp + 1) * 512], ALU.add)
                    at = attp.tile([128, 2 * NQ], BF16, tag="at")
                    nc.scalar.activation(at[:], sc_ps[:], AF.Exp,
                                         bias=b3t128[:, h:h + 1])
                    at_h.append(at)
                for jcp in range(2):
                    nc.tensor.matmul(
                        sums_ps[:], hsel[:, h * 16:(h + 1) * 16],
                        at_h[jcp][:],
                        start=(h == 0 and jcp == 0),
                        stop=(h == 15 and jcp == 1), skip_group_check=True)
                for jc in range(4):
                    st, et = jc, h // 8
                    nc.tensor.matmul(
                        av_ps[hw:hw + 64, :],
                        Vsb[:, (st * 2 + et) * 512 + (h % 8) * 64:
                            (st * 2 + et) * 512 + (h % 8) * 64 + 64],
                        at_h[jc // 2][:, (jc % 2) * NQ:(jc % 2 + 1) * NQ],
                        start=(jc == 0), stop=(jc == 3),
                        skip_group_check=True,
                        tile_position=(0, hw))
                if h % 2 == 1:
                    nc.vector.tensor_copy(kc(avU, hp, NQ), av_tiles[hp][:])

            nc.vector.tensor_copy(sums_h[:], sums_ps[:, 0:NQ])
            nc.vector.tensor_tensor(sums_sb[:], sums_h[:],
                                    sums_ps[:, NQ:2 * NQ], ALU.add)
            with nc.allow_low_precision(reason="f32r is bit-identical f32"):
                nc.vector.reciprocal(recip_sb[:], sums_sb[:])

        # ---------------- phase C: normalize + output projection --------
        with tc.tile_pool(name="osb", bufs=2) as osb, \
             tc.tile_pool(name="r2ps", bufs=2, space="PSUM") as r2ps, \
             tc.tile_pool(name="ops", bufs=2, space="PSUM") as ops:
            for hp in range(8):
                r2 = r2ps.tile([128, NQ], F32, tag="r2")
                nc.tensor.matmul(r2[:], sel8[:, hp * 128:(hp + 1) * 128],
                                 recip_sb[:],
                                 start=True, stop=True)
                nc.vector.tensor_tensor(kc(avN, hp, NQ), kc(avU, hp, NQ),
                                        r2[:], ALU.mult)
            for ttile in range(2):
                for et in range(2):
                    ps = ops.tile([128, 512], F32, tag="ops")
                    for hp in range(8):
                        nc.tensor.matmul(
                            ps[:],
                            kc(avN, hp, NQ)[:, ttile * 128:(ttile + 1) * 128],
                            kc(wo_r, hp, E)[:, et * 512:(et + 1) * 512],
                            start=(hp == 0), stop=(hp == 7))
                    o_sb = osb.tile([128, 512], F32, tag="osb")
                    if (ttile + et) % 2 == 0:
                        nc.scalar.copy(o_sb[:], ps[:])
                    else:
                        nc.vector.tensor_copy(o_sb[:], ps[:])
                    nc.sync.dma_start(
                        out_d[ttile * 128:(ttile + 1) * 128,
                              et * 512:(et + 1) * 512], o_sb[:])

    nc.compile()
    return nc


def _host_prep(inputs):
    """Build the 8 per-core input maps from the full inputs."""
    f32 = np.float32
    q = np.ascontiguousarray(inputs["query"], f32)
    k = np.ascontiguousarray(inputs["key"], f32)
    v = np.ascontiguousarray(inputs["value"], f32)
    se = np.ascontiguousarray(inputs["state_embeddings"], f32)
    scale = f32(D) ** f32(-0.5)
    wq = np.asarray(inputs["Wq"], f32) * scale
    wk = np.asarray(inputs["Wk"], f32)
    wv = np.asarray(inputs["Wv"], f32)
    wo = np.asarray(inputs["Wo"], f32)
    bq = np.asarray(inputs["bq"], f32) * scale
    bk = np.asarray(inputs["bk"], f32)
    bv = np.asarray(inputs["bv"], f32)
    w1 = np.asarray(inputs["W1"], f32)
    b1 = np.asarray(inputs["b1"], f32)
    w2 = np.asarray(inputs["W2"], f32)
    b2 = np.asarray(inputs["b2"], f32)
    w3 = np.asarray(inputs["W3"], f32)
    b3 = np.asarray(inputs["b3"], f32)
    temps = np.asarray(inputs["head_temps"], f32)

    # packed consts
    f32c = np.zeros((128, 34), f32)
    f32c[:, 0:8] = bq.reshape(8, 128).T
    f32c[:, 8:16] = bk.reshape(8, 128).T
    f32c[:, 16] = np.tile(b1, 2)
    f32c[:, 17] = np.tile(b2, 2)
    f32c[:, 18:34] = np.tile((b3 * temps)[None, :], (128, 1))

    w3t = w3 * temps[None, :]
    w2bd = np.zeros((128, 128), f32)
    w2bd[:64, :64] = w2
    w2bd[64:, 64:] = w2
    w3bd = np.zeros((128, 32), f32)
    w3bd[:64, :16] = w3t
    w3bd[64:, 16:] = w3t
    hsel = np.zeros((128, H * 16), f32)
    for h in range(H):
        hsel[:, h * 16 + h] = 1.0
    bf16c = np.zeros((128, 672), f32)
    bf16c[:, 0:128] = np.eye(128, dtype=f32)
    bf16c[:, 128:384] = hsel
    bf16c[:, 384:512] = w2bd
    bf16c[:, 512:544] = w3bd
    # permuted identity: transpose col for src partition (iic, ipar, h)
    # lands at (h, iic, ipar)
    idtr = np.zeros((128, 128), f32)
    for iic in range(4):
        for ipar in range(2):
            for h in range(16):
                idtr[iic * 32 + ipar * 16 + h, h * 8 + iic * 2 + ipar] = 1.0
    bf16c[:, 544:672] = idtr

    w1ab = np.concatenate(
        [np.tile(w1[:SD], (1, 2)), np.tile(w1[SD:], (1, 2))],
        axis=1)                                             # [64, 256]

    sel8 = np.zeros((16, 1024), f32)
    for hp in range(8):
        sel8[2 * hp, hp * 128:hp * 128 + 64] = 1.0
        sel8[2 * hp + 1, hp * 128 + 64:hp * 128 + 128] = 1.0

    onespk = np.zeros((1, 128 + E), f32)
    onespk[0, :128] = 1.0
    onespk[0, 128:] = bv

    def packw(w):  # [in, out] f32 -> [ec*128+p, k*128+c] bf16
        return np.ascontiguousarray(
            w.reshape(8, 128, 8, 128).transpose(2, 1, 0, 3)
            .reshape(E, E)).astype(_BF)

    shared = dict(f32c=f32c, bf16c=bf16c.astype(_BF), w1ab=w1ab.astype(_BF),
                  sel8=sel8, onespk=onespk.astype(_BF),
                  wqp=packw(wq), wkp=packw(wk),
                  wv=wv.astype(_BF), wo=wo.astype(_BF))
    maps = []
    for c in range(NCORES):
        b, half = c // 2, c % 2
        rows = slice(half * NQ, (half + 1) * NQ)
        m = dict(shared)
        m["qT"] = np.ascontiguousarray(q[b, rows].T).astype(_BF)
        m["kT"] = np.ascontiguousarray(k[b].T).astype(_BF)
        vT = v[b].T                                          # [E, N]
        m["vtp"] = np.ascontiguousarray(
            vT.reshape(8, 128, 4, 128).transpose(2, 1, 0, 3)
            .reshape(N, E)).astype(_BF)
        m["seT"] = np.ascontiguousarray(se[b].T).astype(_BF)
        m["seQ"] = np.ascontiguousarray(se[b, rows].T).astype(_BF)
        maps.append(m)
    return maps


_cache = {}


def _get_program():
    if "nc" not in _cache:
        _cache["nc"] = _build_program()
    return _cache["nc"]


def kernel(**inputs):
    nc = _get_program()
    maps = _host_prep(inputs)
    res = run_bass_kernel_spmd(nc, maps, list(range(NCORES)))
    bo = np.asarray(inputs["bo"], np.float32)
    out = np.empty((B, N, E), np.float32)
    for c in range(NCORES):
        b, half = c // 2, c % 2
        out[b, half * NQ:(half + 1) * NQ] = res.results[c]["out"]
    return out + bo


# revision 17
# speedup vs baseline: 1.0462x; 1.0462x over previous
"""CAB multi-head attention on 8 Trainium2 NeuronCores.

Sharding: fully data-parallel, core c -> (batch b = c//2, query-half = c%2).
Each core computes 256 query rows against all 512 keys of its batch.
No collectives. Host does transposes/packing; device does all FLOPs.

Per-core layout conventions (features on partitions, tokens on free):
  QT/KT [E, t] bf16; V [s, e] bf16; scoresT/attnT [s, t] (softmax along
  partitions via one-hot-column matmuls, no max subtraction needed);
  CAB pairs i-major: h/h2 [(d, i%2), j]; comp [(iic, i%2+h), j] is
  PE-transposed into biasT [j, (tt, jc, c)] and pre-loaded into the
  scores PSUM via an identity matmul with a strided moving AP.

All matmuls bf16 (fp32 PSUM accumulate). head_temps folded into W3 on
host; b3*temps added via the exp activation bias. Projection weights
host-packed so every DMA is contiguous. Phase-1 projections interleaved
into the CAB tt loop so PE/DVE/Scalar all stay busy while weights
stream in.
"""
import sys

sys.path.insert(0, "/opt/trn_rl_repo")

import numpy as np
import ml_dtypes
from contextlib import ExitStack

import concourse.bacc as bacc
import concourse.tile as tile
from concourse import mybir
from concourse.bass_utils import run_bass_kernel_spmd

F32 = mybir.dt.float32
F32R = mybir.dt.float32r
BF16 = mybir.dt.bfloat16
AF = mybir.ActivationFunctionType
ALU = mybir.AluOpType

B, N, E, H, SD, HID = 4, 512, 1024, 16, 64, 64
D = E // H
NQ = 256            # query rows per core
NCORES = 8
NTT = NQ // 8       # 32 tt groups (4 i-pairs each) in the CAB stage

_BF = ml_dtypes.bfloat16


def _build_program():
    nc = bacc.Bacc("TRN2", target_bir_lowering=False, debug=False,
                   num_devices=NCORES)

    def din(name, shape, dt):
        return nc.dram_tensor(name, list(shape), dt, kind="ExternalInput").ap()

    d = {}
    # packed consts
    d["f32c"] = din("f32c", (128, 34), F32)      # bq(8) bk(8) b1(1) b2(1) b3t(16)
    d["bf16c"] = din("bf16c", (128, 672), BF16)  # id128 hsel w2bd w3bd idtr
    d["w1ab"] = din("w1ab", (SD, 256), BF16)     # w1a | w1b
    d["seT"] = din("seT", (SD, N), BF16)
    d["seQ"] = din("seQ", (SD, NQ), BF16)
    d["sel8"] = din("sel8", (16, 1024), F32R)    # recip row-broadcast selectors
    d["onespk"] = din("onespk", (1, 128 + E), BF16)  # ones(128) | bv
    # per-core activations
    d["qT"] = din("qT", (E, NQ), BF16)
    d["kT"] = din("kT", (E, NQ), BF16)           # this core's key half
    d["vtp"] = din("vtp", (NQ, E), BF16)         # packed V-proj stationary (half)
    # weights (packed/bf16)
    d["wqp"] = din("wqp", (E, E), BF16)
    d["wkp"] = din("wkp", (E, E), BF16)
    d["wv"] = din("wv", (E, E), BF16)
    d["wo"] = din("wo", (E, E), BF16)
    out_d = nc.dram_tensor("out", [NQ, E], F32, kind="ExternalOutput").ap()
    kexch = nc.dram_tensor("kexch", [E, NQ], BF16).ap()
    vexch = nc.dram_tensor("vexch", [NQ, E], BF16).ap()
    kgath = nc.dram_tensor("kgath", [2 * E, NQ], BF16).ap()
    vgath = nc.dram_tensor("vgath", [N, E], BF16).ap()
    PAIRS = [[0, 1], [2, 3], [4, 5], [6, 7]]

    with tile.TileContext(nc) as tc, ExitStack() as ctx:
        # ---------------- persistent SBUF ----------------
        cst = ctx.enter_context(tc.tile_pool(name="cst", bufs=1))
        big = ctx.enter_context(tc.tile_pool(name="big", bufs=1))

        def cload(name, shape, dt, src=None):
            t = cst.tile(list(shape), dt, tag=name, name=name)
            nc.sync.dma_start(t[:], (src if src is not None else d[name][:]))
            return t

        # small consts first (CAB needs them immediately)
        f32c = cload("f32c", (128, 34), F32)
        bq128 = f32c[:, 0:8]
        bk128 = f32c[:, 8:16]
        b1d = f32c[:, 16:17]
        b2d = f32c[:, 17:18]
        b3t128 = f32c[:, 18:34]
        bf16c = cload("bf16c", (128, 672), BF16)
        id128 = bf16c[:, 0:128]
        hsel = bf16c[:, 128:384]
        w2bd = bf16c[:, 384:512]
        w3bd = bf16c[:, 512:544]
        idtr = bf16c[:, 544:672]
        w1ab = cload("w1ab", (SD, 256), BF16)
        seQ = cload("seQ", (SD, NQ), BF16)
        seT = cload("seT", (SD, N), BF16)
        sel8 = cload("sel8", (16, 1024), F32R)
        onespk = cload("onespk", (1, 128 + E), BF16)
        ones1 = onespk[:, 0:128]
        bv2d = onespk[:, 128:128 + E]

        # big per-core inputs / weights, one contiguous DMA each, in
        # priority order (CAB first, out-proj weights last).
        def kview(name, t, chunk_elems):
            tt = big.tile([128, t], BF16, tag=name, name=name)
            nc.sync.dma_start(
                tt[:], d[name][:].rearrange("(k p) t -> p k t", p=128))
            return tt

        kTt = kview("kT", 8 * NQ, NQ)          # [128, (k s-half)]
        vtp = kview("vtp", 2 * E, E)           # [128, (st_l kc)]
        qTt = kview("qT", 8 * NQ, NQ)          # [128, (k t)]
        wv_r = kview("wv", 8 * E, E)
        wkp = kview("wkp", 8 * E, E)
        wqp = kview("wqp", 8 * E, E)
        wo_r = kview("wo", 8 * E, E)

        def kc(tbl, k, w):
            return tbl[:, k * w:(k + 1) * w]

        # persistent intermediates
        QT = big.tile([128, 8 * NQ], BF16, tag="QT")
        KT = big.tile([128, 8 * N], BF16, tag="KT")
        KTh = big.tile([128, 8 * NQ], BF16, tag="KTh")    # this half only
        Vsb = big.tile([128, 8 * 512], BF16, tag="Vsb")   # (st et) chunks
        Vsb_h = big.tile([128, 4 * 512], BF16, tag="Vsb_h")
        hjT = big.tile([128, N], BF16, tag="hjT")
        hiT = big.tile([128, 128], F32, tag="hiT")
        biasT = big.tile([128, NTT * 512], BF16, tag="biasT")
        avU = big.tile([128, 8 * NQ], F32, tag="avU")
        avN = big.tile([128, 8 * NQ], BF16, tag="avN")
        sums_h = big.tile([16, NQ], F32, tag="sums_h")
        sums_sb = big.tile([16, NQ], F32, tag="sums_sb")
        recip_sb = big.tile([16, NQ], F32R, tag="recip_sb")

        # ---------------- phase A: CAB MLP with projections woven in ----
        with tc.tile_pool(name="hpool", bufs=6) as hpool, \
             tc.tile_pool(name="h2sb", bufs=3) as h2sbp, \
             tc.tile_pool(name="csb", bufs=3) as csbp, \
             tc.tile_pool(name="w2ps", bufs=2, space="PSUM") as w2ps, \
             tc.tile_pool(name="cps", bufs=2, space="PSUM") as cps, \
             tc.tile_pool(name="trps", bufs=1, space="PSUM") as trps, \
             tc.tile_pool(name="p1ps", bufs=1, space="PSUM") as p1ps:

            # W1: hjT (dup'd, +b1, bf16) and hiT (packed by i-parity, f32)
            hj_ps = p1ps.tile([128, N], F32, tag="p1")
            nc.tensor.matmul(hj_ps[:], w1ab[:, 128:256], seT[:],
                             start=True, stop=True)
            nc.vector.tensor_scalar(hjT[:], hj_ps[:], b1d[:, 0:1], None,
                                    ALU.add)
            hi_ps = p1ps.tile([128, N], F32, tag="p1", name="hi_ps")[:, 0:NQ]
            nc.tensor.matmul(hi_ps[:], w1ab[:, 0:128], seQ[:],
                             start=True, stop=True)
            hi_v = hi_ps[:].rearrange("p (i two) -> p i two", two=2)
            nc.vector.tensor_copy(hiT[0:64, :], hi_v[0:64, :, 0])
            nc.vector.tensor_copy(hiT[64:128, :], hi_v[64:128, :, 1])

            # phase-1 chunk emitters (interleaved into the tt loop)
            def v_chunk(i):
                st_l, et = i // 2, i % 2
                ps = p1ps.tile([128, N], F32, tag="p1")
                for k in range(8):
                    nc.tensor.matmul(
                        ps[:], kc(vtp, st_l, E)[:, k * 128:(k + 1) * 128],
                        kc(wv_r, k, E)[:, et * 512:(et + 1) * 512],
                        start=(k == 0), stop=False)
                nc.tensor.matmul(ps[:], ones1[0:1, 0:128],
                                 bv2d[0:1, et * 512:(et + 1) * 512],
                                 start=False, stop=True)
                nc.scalar.copy(Vsb_h[:, (st_l * 2 + et) * 512:
                                     (st_l * 2 + et + 1) * 512], ps[:])

            def k_chunk(ec):
                ps = p1ps.tile([128, N], F32, tag="p1", name="kps")[:, 0:NQ]
                for k in range(8):
                    nc.tensor.matmul(
                        ps[:], kc(wkp, ec, E)[:, k * 128:(k + 1) * 128],
                        kc(kTt, k, NQ), start=(k == 0), stop=(k == 7))
                nc.vector.tensor_scalar(kc(KTh, ec, NQ), ps[:],
                                        bk128[:, ec:ec + 1], None, ALU.add)

            def kv_exchange():
                nc.sync.dma_start(
                    kexch[:].rearrange("(k p) s -> p k s", p=128), KTh[:])
                nc.sync.dma_start(
                    vexch[:].rearrange("(s2 p) e -> p s2 e", p=128),
                    Vsb_h[:])
                nc.gpsimd.collective_compute(
                    "AllGather", ALU.bypass, replica_groups=PAIRS,
                    ins=[kexch[:]], outs=[kgath[:]])
                nc.gpsimd.collective_compute(
                    "AllGather", ALU.bypass, replica_groups=PAIRS,
                    ins=[vexch[:]], outs=[vgath[:]])
                for g in range(2):
                    nc.sync.dma_start(
                        KT[:].rearrange("p (k g s) -> p k g s",
                                        k=8, g=2)[:, :, g, :],
                        kgath[g * E:(g + 1) * E, :].rearrange(
                            "(k p) s -> p k s", p=128))
                nc.sync.dma_start(
                    Vsb[:].rearrange("p (st e) -> p st e", st=4),
                    vgath[:].rearrange("(st p) e -> p st e", p=128))

            def q_chunk(ec):
                ps = p1ps.tile([128, N], F32, tag="p1", name="qps")[:, 0:NQ]
                for k in range(8):
                    nc.tensor.matmul(
                        ps[:], kc(wqp, ec, E)[:, k * 128:(k + 1) * 128],
                        kc(qTt, k, NQ), start=(k == 0), stop=(k == 7))
                nc.vector.tensor_scalar(kc(QT, ec, NQ), ps[:],
                                        bq128[:, ec:ec + 1], None, ALU.add)

            sched = {}
            for n, i in enumerate(range(4)):          # V at tt 2..5
                sched.setdefault(2 + n, []).append(("v", i))
            for n, ec in enumerate(range(8)):         # K at tt 7,9..21
                sched.setdefault(7 + 2 * n, []).append(("k", ec))
            sched.setdefault(22, []).append(("x", 0))
            for n, ec in enumerate(range(8)):         # Q at tt 24..31
                sched.setdefault(24 + n, []).append(("q", ec))

            for tt in range(NTT):
                h_tiles = []
                for iic in range(4):
                    ii = tt * 4 + iic
                    h_t = hpool.tile([128, N], BF16, tag="h")
                    nc.vector.tensor_scalar(h_t[:], hjT[:],
                                            hiT[:, ii:ii + 1], 0.0,
                                            ALU.add, ALU.max)
                    h_tiles.append(h_t)
                h2_tiles = []
                for half in range(2):
                    ps2 = w2ps.tile([128, 2 * N], F32, tag="h2")
                    for j in range(2):
                        nc.tensor.matmul(ps2[:, j * N:(j + 1) * N], w2bd[:],
                                         h_tiles[half * 2 + j][:],
                                         start=True, stop=True)
                    h2_t = h2sbp.tile([128, 2 * N], BF16, tag="h2sb")
                    nc.scalar.activation(h2_t[:], ps2[:], AF.Relu,
                                         bias=b2d[:, 0:1])
                    h2_tiles.append(h2_t)

                c_ps = cps.tile([128, N], F32, tag="comp")
                for iic in range(4):
                    nc.tensor.matmul(c_ps[32 * iic:32 * iic + 32, :],
                                     w3bd[:],
                                     h2_tiles[iic // 2][:, (iic % 2) * N:
                                                        (iic % 2 + 1) * N],
                                     start=True, stop=True,
                                     tile_position=(0, 32 * iic))
                c_sb = csbp.tile([128, N], BF16, tag="csb")
                if tt % 2 == 0:
                    nc.vector.tensor_copy(c_sb[:], c_ps[:])
                else:
                    nc.scalar.copy(c_sb[:], c_ps[:])
                tr_ps = trps.tile([128, 512], BF16, tag="tr")
                for jc in range(4):
                    nc.tensor.transpose(tr_ps[:, jc * 128:(jc + 1) * 128],
                                        c_sb[:, jc * 128:(jc + 1) * 128],
                                        idtr[:])
                # tr_ps cols are (jc, h, iic, ipar) via the permuted
                # identity; copy into h-major biasT (col = h*1024 +
                # jc*256 + t) so phase-B preloads are contiguous.
                bdst = biasT[:].rearrange(
                    "p (x j to i m) -> p j x i m to",
                    x=16, j=4, to=NTT, i=4, m=2)[:, :, :, :, :, tt]
                bsrc = tr_ps[:].rearrange(
                    "p (j x i m) -> p j x i m", j=4, x=16, i=4, m=2)
                nc.vector.tensor_copy(bdst, bsrc)

                for kind, i in sched.get(tt, []):
                    (v_chunk if kind == "v" else
                     k_chunk if kind == "k" else
                     q_chunk if kind == "q" else
                     (lambda _i: kv_exchange()))(i)

        # ---------------- phase B: scores + softmax + AV ----------------
        with tc.tile_pool(name="attnT", bufs=8) as attp, \
             tc.tile_pool(name="scps", bufs=5, space="PSUM") as scps, \
             tc.tile_pool(name="smps", bufs=1, space="PSUM") as smps, \
             tc.tile_pool(name="avps", bufs=2, space="PSUM") as avps:

            sums_ps = smps.tile([16, 2 * NQ], F32, tag="sums")
            av_tiles = [None] * 8
            for h in range(16):
                hp, hw = h // 2, (h % 2) * 64
                if h % 2 == 0:
                    av_ps = avps.tile([128, NQ], F32, tag="av")
                    av_tiles[hp] = av_ps
                at_h = []
                for jcp in range(2):           # two j-chunks per tile
                    sc_ps = scps.tile([128, 2 * NQ], F32, tag="sc")
                    for jh in range(2):
                        jc = jcp * 2 + jh
                        nc.tensor.matmul(
                            sc_ps[:, jh * NQ:(jh + 1) * NQ],
                            kc(KT, hp, N)[hw:hw + 64,
                                          jc * 128:(jc + 1) * 128],
                            kc(QT, hp, NQ)[hw:hw + 64, :],
                            start=True, stop=True, skip_group_check=True)
                    # bias add on DVE (PE stays matmul-only); exp reads f32
                    nc.vector.tensor_tensor(
                        sc_ps[:], sc_ps[:],
                        biasT[:, h * 1024 + jcp * 512:
                              h * 1024 + (jcp + 1) * 512], ALU.add)
                    at = attp.tile([128, 2 * NQ], BF16, tag="at")
                    nc.scalar.activation(at[:], sc_ps[:], AF.Exp,
                                         bias=b3t128[:, h:h + 1])
                    at_h.append(at)
                for jcp in range(2):
                    nc.tensor.matmul(
                        sums_ps[:], hsel[:, h * 16:(h + 1) * 16],
                        at_h[jcp][:],
                        start=(h == 0 and jcp == 0),
                        stop=(h == 15 and jcp == 1), skip_group_check=True)
                for jc in range(4):
                    st, et = jc, h // 8
                    nc.tensor.matmul(
                        av_ps[hw:hw + 64, :],
                        Vsb[:, (st * 2 + et) * 512 + (h % 8) * 64:
                            (st * 2 + et) * 512 + (h % 8) * 64 + 64],
                        at_h[jc // 2][:, (jc % 2) * NQ:(jc % 2 + 1) * NQ],
                        start=(jc == 0), stop=(jc == 3),
                        skip_group_check=True,
                        tile_position=(0, hw))
                if h % 2 == 1:
                    nc.vector.tensor_copy(kc(avU, hp, NQ), av_tiles[hp][:])

            nc.vector.tensor_copy(sums_h[:], sums_ps[:, 0:NQ])
            nc.vector.tensor_tensor(sums_sb[:], sums_h[:],
                                    sums_ps[:, NQ:2 * NQ], ALU.add)
            with nc.allow_low_precision(reason="f32r is bit-identical f32"):
                nc.vector.reciprocal(recip_sb[:], sums_sb[:])

        # ---------------- phase C: normalize + output projection --------
        with tc.tile_pool(name="osb", bufs=2) as osb, \
             tc.tile_pool(name="r2ps", bufs=2, space="PSUM") as r2ps, \
             tc.tile_pool(name="ops", bufs=2, space="PSUM") as ops:
            for hp in range(8):
                r2 = r2ps.tile([128, NQ], F32, tag="r2")
                nc.tensor.matmul(r2[:], sel8[:, hp * 128:(hp + 1) * 128],
                                 recip_sb[:],
                                 start=True, stop=True)
                nc.vector.tensor_tensor(kc(avN, hp, NQ), kc(avU, hp, NQ),
                                        r2[:], ALU.mult)
            for ttile in range(2):
                for et in range(2):
                    ps = ops.tile([128, 512], F32, tag="ops")
                    for hp in range(8):
                        nc.tensor.matmul(
                            ps[:],
                            kc(avN, hp, NQ)[:, ttile * 128:(ttile + 1) * 128],
                            kc(wo_r, hp, E)[:, et * 512:(et + 1) * 512],
                            start=(hp == 0), stop=(hp == 7))
                    o_sb = osb.tile([128, 512], F32, tag="osb")
                    if (ttile + et) % 2 == 0:
                        nc.scalar.copy(o_sb[:], ps[:])
                    else:
                        nc.vector.tensor_copy(o_sb[:], ps[:])
                    nc.sync.dma_start(
                        out_d[ttile * 128:(ttile + 1) * 128,
                              et * 512:(et + 1) * 512], o_sb[:])

    nc.compile()
    return nc


def _host_prep(inputs):
    """Build the 8 per-core input maps from the full inputs."""
    f32 = np.float32
    q = np.ascontiguousarray(inputs["query"], f32)
    k = np.ascontiguousarray(inputs["key"], f32)
    v = np.ascontiguousarray(inputs["value"], f32)
    se = np.ascontiguousarray(inputs["state_embeddings"], f32)
    scale = f32(D) ** f32(-0.5)
    wq = np.asarray(inputs["Wq"], f32) * scale
    wk = np.asarray(inputs["Wk"], f32)
    wv = np.asarray(inputs["Wv"], f32)
    wo = np.asarray(inputs["Wo"], f32)
    bq = np.asarray(inputs["bq"], f32) * scale
    bk = np.asarray(inputs["bk"], f32)
    bv = np.asarray(inputs["bv"], f32)
    w1 = np.asarray(inputs["W1"], f32)
    b1 = np.asarray(inputs["b1"], f32)
    w2 = np.asarray(inputs["W2"], f32)
    b2 = np.asarray(inputs["b2"], f32)
    w3 = np.asarray(inputs["W3"], f32)
    b3 = np.asarray(inputs["b3"], f32)
    temps = np.asarray(inputs["head_temps"], f32)

    # packed consts
    f32c = np.zeros((128, 34), f32)
    f32c[:, 0:8] = bq.reshape(8, 128).T
    f32c[:, 8:16] = bk.reshape(8, 128).T
    f32c[:, 16] = np.tile(b1, 2)
    f32c[:, 17] = np.tile(b2, 2)
    f32c[:, 18:34] = np.tile((b3 * temps)[None, :], (128, 1))

    w3t = w3 * temps[None, :]
    w2bd = np.zeros((128, 128), f32)
    w2bd[:64, :64] = w2
    w2bd[64:, 64:] = w2
    w3bd = np.zeros((128, 32), f32)
    w3bd[:64, :16] = w3t
    w3bd[64:, 16:] = w3t
    hsel = np.zeros((128, H * 16), f32)
    for h in range(H):
        hsel[:, h * 16 + h] = 1.0
    bf16c = np.zeros((128, 672), f32)
    bf16c[:, 0:128] = np.eye(128, dtype=f32)
    bf16c[:, 128:384] = hsel
    bf16c[:, 384:512] = w2bd
    bf16c[:, 512:544] = w3bd
    # permuted identity: transpose col for src partition (iic, ipar, h)
    # lands at (h, iic, ipar)
    idtr = np.zeros((128, 128), f32)
    for iic in range(4):
        for ipar in range(2):
            for h in range(16):
                idtr[iic * 32 + ipar * 16 + h, h * 8 + iic * 2 + ipar] = 1.0
    bf16c[:, 544:672] = idtr

    w1ab = np.concatenate(
        [np.tile(w1[:SD], (1, 2)), np.tile(w1[SD:], (1, 2))],
        axis=1)                                             # [64, 256]

    sel8 = np.zeros((16, 1024), f32)
    for hp in range(8):
        sel8[2 * hp, hp * 128:hp * 128 + 64] = 1.0
        sel8[2 * hp + 1, hp * 128 + 64:hp * 128 + 128] = 1.0

    onespk = np.zeros((1, 128 + E), f32)
    onespk[0, :128] = 1.0
    onespk[0, 128:] = bv

    def packw(w):  # [in, out] f32 -> [ec*128+p, k*128+c] bf16
        return np.ascontiguousarray(
            w.reshape(8, 128, 8, 128).transpose(2, 1, 0, 3)
            .reshape(E, E)).astype(_BF)

    shared = dict(f32c=f32c, bf16c=bf16c.astype(_BF), w1ab=w1ab.astype(_BF),
                  sel8=sel8, onespk=onespk.astype(_BF),
                  wqp=packw(wq), wkp=packw(wk),
                  wv=wv.astype(_BF), wo=wo.astype(_BF))
    maps = []
    for c in range(NCORES):
        b, half = c // 2, c % 2
        rows = slice(half * NQ, (half + 1) * NQ)
        m = dict(shared)
        m["qT"] = np.ascontiguousarray(q[b, rows].T).astype(_BF)
        m["kT"] = np.ascontiguousarray(k[b, rows].T).astype(_BF)
        vTh = v[b, rows].T                                   # [E, NQ]
        m["vtp"] = np.ascontiguousarray(
            vTh.reshape(8, 128, 2, 128).transpose(2, 1, 0, 3)
            .reshape(NQ, E)).astype(_BF)
        m["seT"] = np.ascontiguousarray(se[b].T).astype(_BF)
        m["seQ"] = np.ascontiguousarray(se[b, rows].T).astype(_BF)
        maps.append(m)
    return maps


_cache = {}


def _get_program():
    if "nc" not in _cache:
        _cache["nc"] = _build_program()
    return _cache["nc"]


def kernel(**inputs):
    nc = _get_program()
    maps = _host_prep(inputs)
    res = run_bass_kernel_spmd(nc, maps, list(range(NCORES)))
    bo = np.asarray(inputs["bo"], np.float32)
    out = np.empty((B, N, E), np.float32)
    for c in range(NCORES):
        b, half = c // 2, c % 2
        out[b, half * NQ:(half + 1) * NQ] = res.results[c]["out"]
    return out + bo
